# revision 21
# baseline (speedup 1.0000x reference)
"""Trainium2 kernel for nn_CustomizedMoGPositionwiseFF (moe_routing).

Strategy (expert-parallel, per the sharding hint):
  - 32 (group, expert) FFN pairs are sharded across 8 NeuronCores (4 each).
  - Routing (group top-2 gate + per-group inner top-2 gate) is computed on
    host at call time; tokens are dispatched (gathered) per expert into the
    per-core shards -- data-dependent sharding, compiled into the NEFF.
  - Each core runs both FFN matmuls + relu for its 4 experts over the tokens
    routed to them, reading each expert weight exactly once (memory regime).
    Weights and activations are shipped as fp8 e4m3 and the matmuls run in
    DoubleRow perf mode (2 fp8 weights per PE cell, 256-deep contraction):
    ~2x the bf16 PE throughput and half the weight DMA traffic.  PSUM
    accumulation stays f32; relative error vs the f32 reference ~4e-3.
  - Host applies the cheap O(N*D) combine: iw/b2 scaling, scatter-add of the
    two expert contributions per (token, group), per-group post-layernorm,
    group top-2 mixture, and the outer residual.

The kernel output layout on device is u^T = (relu(z W1 + b1) W2)^T per
dispatched token, written as [128, DT, CT] bf16 so every DMA is dense.
"""

import os
import numpy as np

# Model dims (hardcoded per the contract; match the reference problem)
B, T, D, H = 2, 1024, 512, 2048
G, E, GK, EK = 4, 8, 2, 2
EPS = 1e-5
N = B * T
P = 128
DT = D // P    # 4 d-tiles
HT = H // P    # 16 h-tiles
NCORES = 8
SLOTS = (G * E) // NCORES  # 4 experts per core
CAP_GRAN = 16              # capacity granularity (tokens)

_nc_cache = {}
LAST_RESULTS = None       # test harness can inspect (BassKernelResults)


def _ensure_ntff_hook():
    """Register antenv.axon_hooks with the ctypes NTFF profile hook if the
    container's antenv package lacks it (mirrors trn_agent_boot.trn_boot).
    Makes trace=True work; degrades to hook=None when the .so is absent."""
    try:
        from antenv.axon_hooks import get_axon_ntff_profile_hook  # noqa: F401
        return
    except ImportError:
        pass
    import sys
    import types
    import contextlib
    import ctypes

    mod = types.ModuleType("antenv.axon_hooks")
    _state = {"hook": None}

    def set_axon_ntff_profile_hook(h):
        _state["hook"] = h

    def get_axon_ntff_profile_hook():
        return _state["hook"]

    mod.set_axon_ntff_profile_hook = set_axon_ntff_profile_hook
    mod.get_axon_ntff_profile_hook = get_axon_ntff_profile_hook

    so_path = "/opt/axon/libaxon_pjrt.so"
    hook = None
    if os.path.exists(so_path):
        try:
            lib = ctypes.CDLL(so_path)
            if hasattr(lib, "axon_start_nrt_profile"):
                lib.axon_start_nrt_profile.argtypes = [
                    ctypes.POINTER(ctypes.c_int64), ctypes.c_size_t]
                lib.axon_start_nrt_profile.restype = ctypes.c_int64
                lib.axon_stop_nrt_profile.argtypes = [ctypes.c_char_p]
                lib.axon_stop_nrt_profile.restype = ctypes.c_int64

                @contextlib.contextmanager
                def _hook(output_dir, device_ids):
                    import jax
                    jax.devices()
                    if device_ids:
                        ids = (ctypes.c_int64 * len(device_ids))(*device_ids)
                        rc = lib.axon_start_nrt_profile(ids, len(device_ids))
                    else:
                        rc = lib.axon_start_nrt_profile(None, 0)
                    if rc != 0:
                        raise RuntimeError(f"axon_start_nrt_profile rc={rc}")
                    try:
                        yield
                    finally:
                        n = lib.axon_stop_nrt_profile(str(output_dir).encode())
                        print(f"ntff profile: {n} file(s) -> {output_dir}")

                hook = _hook
        except Exception:
            hook = None
    _state["hook"] = hook
    import antenv
    sys.modules["antenv.axon_hooks"] = mod
    antenv.axon_hooks = mod


def _round_up(x, m):
    return ((x + m - 1) // m) * m


def _routing(inp, ln_g, ln_b, wg_group, wg_inner):
    """Replicate the reference gating bit-for-bit on jax-cpu.

    Returns gi [N,GK] group ids, gsc [N,GK] group softmax, z [N,D] f32,
    eis/escs: per-group inner top-k ids/softmax ([N,EK] each).
    """
    import jax
    import jax.numpy as jnp

    cpu = jax.devices("cpu")[0]
    with jax.default_device(cpu):
        x = jnp.asarray(np.asarray(inp, np.float32)).reshape(-1, D)
        gl = x @ jnp.asarray(np.asarray(wg_group, np.float32))
        gv, gi = jax.lax.top_k(gl, GK)
        gsc = jax.nn.softmax(gv, axis=-1)
        m = jnp.mean(x, axis=-1, keepdims=True)
        xc = x - m
        v = jnp.mean(xc * xc, axis=-1, keepdims=True)
        z = xc * jax.lax.rsqrt(v + EPS) * jnp.asarray(np.asarray(ln_g, np.float32)) \
            + jnp.asarray(np.asarray(ln_b, np.float32))
        wgi = jnp.asarray(np.asarray(wg_inner, np.float32))
        eis, escs = [], []
        for g in range(G):
            l = z @ wgi[g]
            ev, ei = jax.lax.top_k(l, EK)
            esc = jax.nn.softmax(ev, axis=-1)
            eis.append(np.asarray(ei))
            escs.append(np.asarray(esc))
    return np.asarray(gi), np.asarray(gsc), np.asarray(z), eis, escs


def _build_nc(Cs, has_b1=False):
    """Build the SPMD Bass program for per-slot capacities Cs (uniform across cores).

    fp8 e4m3 weights + activations, DoubleRowSwInterleave matmuls (256-deep
    contraction; weights pre-interleaved on host so the stationary load is a
    dense 256-column read).
    """
    import concourse.bass as bass
    import concourse.bacc as bacc
    import concourse.tile as tile
    from concourse import mybir

    f32 = mybir.dt.float32
    bf16 = mybir.dt.bfloat16
    f8 = mybir.dt.float8e4
    DR = mybir.MatmulPerfMode.DoubleRowSwInterleave
    Relu = mybir.ActivationFunctionType.Relu
    Copy = mybir.ActivationFunctionType.Copy

    CT = int(sum(Cs))
    offs = np.concatenate([[0], np.cumsum(Cs)]).astype(int)
    CMAX = int(max(Cs))

    nc = bacc.Bacc("TRN2", target_bir_lowering=False)
    # all DRAM layouts are partition-major [128, ...] so every DMA is dense
    # contiguous lines per partition.  Weight tiles are stored as the PE's
    # SwInterleave stationary layout: per (pair, out-tile) a contiguous 256
    # columns [A127 B127 A126 B126 .. A0 B0].
    zt_d = [nc.declare_dram_parameter(f"zt{s}", [P, DT, int(Cs[s])], f8,
                                      isOutput=False) for s in range(SLOTS)]
    w1_d = nc.declare_dram_parameter("w1", [SLOTS, P, HT, DT // 2, 2 * P], f8, isOutput=False)
    w2_d = nc.declare_dram_parameter("w2", [SLOTS, P, DT, HT // 2, 2 * P], f8, isOutput=False)
    b1_d = nc.declare_dram_parameter("b1", [P, SLOTS * HT], f32, isOutput=False)
    u_d = nc.declare_dram_parameter("u", [P, DT, CT], bf16, isOutput=True)

    with tile.TileContext(nc) as tc:
        with tc.tile_pool(name="consts", bufs=1) as consts, \
             tc.tile_pool(name="hpool", bufs=2) as hpool, \
             tc.tile_pool(name="hpsum", bufs=3, space="PSUM") as hpsum, \
             tc.tile_pool(name="upsum", bufs=2, space="PSUM") as upsum, \
             tc.tile_pool(name="usb", bufs=3) as usb:

            zt_sb = [consts.tile([P, DT, int(Cs[s])], f8, tag=f"zt{s}",
                                 name=f"zt_{s}") for s in range(SLOTS)]
            b1_sb = consts.tile([P, SLOTS * HT], f32, tag="b1")
            zero_sb = consts.tile([P, 2, CMAX], f32, tag="zero")
            dummy_sb = consts.tile([P, 512], f8, tag="dummy")
            dscr_sb = consts.tile([P, 256], f8, tag="dscr")
            nc.vector.memset(dummy_sb[:, :], 0.0)
            nc.vector.memset(zero_sb[:, :, :], 0.0)
            # Weight tiles are split into independently-loaded pieces (Tile
            # tracks dependencies at tile granularity, so each piece must be
            # its own tile for compute to start before the whole weight set
            # lands).  w1: ht-ranges; w2: dt-ranges.
            w1_split = [(0, 4), (4, 8), (8, 16)]
            w2_split = [(0, 2), (2, 4)]
            w1_sb = [[consts.tile([P, b - a, DT // 2, 2 * P], f8,
                                  tag=f"w1_{s}_{a}", name=f"w1s_{s}_{a}")
                      for (a, b) in w1_split] for s in range(SLOTS)]
            w2_sb = [[consts.tile([P, b - a, HT // 2, 2 * P], f8,
                                  tag=f"w2_{s}_{a}", name=f"w2s_{s}_{a}")
                      for (a, b) in w2_split] for s in range(SLOTS)]

            def w1_ap(s, ht):
                for i, (a, b) in enumerate(w1_split):
                    if a <= ht < b:
                        return w1_sb[s][i][:, ht - a, :, :]

            def w2_ap(s, dt):
                for i, (a, b) in enumerate(w2_split):
                    if a <= dt < b:
                        return w2_sb[s][i][:, dt - a, :, :]

            def w1_load(eng, s, piece):
                a, b = w1_split[piece]
                eng.dma_start(w1_sb[s][piece][:, :, :, :], w1_d[s][:, a:b, :, :])

            def w2_load(eng, s, piece):
                a, b = w2_split[piece]
                eng.dma_start(w2_sb[s][piece][:, :, :, :], w2_d[s][:, a:b, :, :])

            # ---- resident loads.  The two HWDGE rings (Sync and Scalar)
            # each sustain only ~200 GB/s, so the ~9MB of weights is split
            # ~50/50.  The Sync engine is free until the final output DMAs,
            # so its whole schedule is queued up front; the Scalar engine
            # also runs ACT relu/copy work, so only its first few transfers
            # are queued here and the rest are interleaved into the compute
            # stream (emitted between ACTIVATEs) to keep its ring shallow.
            nc.sync.dma_start(zt_sb[0][:, :, :], zt_d[0][:, :, :])
            nc.sync.dma_start(b1_sb[:, :], b1_d[:, :])
            w1_load(nc.sync, 0, 0)
            w1_load(nc.sync, 0, 1)
            nc.scalar.dma_start(zt_sb[1][:, :, :], zt_d[1][:, :, :])
            w1_load(nc.scalar, 0, 2)
            w2_load(nc.scalar, 0, 1)
            nc.sync.dma_start(zt_sb[2][:, :, :], zt_d[2][:, :, :])
            w2_load(nc.sync, 0, 0)
            w1_load(nc.sync, 1, 0)
            w1_load(nc.sync, 1, 1)
            w2_load(nc.sync, 1, 0)
            w1_load(nc.sync, 2, 0)
            w2_load(nc.sync, 2, 0)
            w1_load(nc.sync, 3, 0)
            w2_load(nc.sync, 3, 0)
            # scalar's remaining transfers, emitted later between ACTIVATEs:
            scalar_late = [
                lambda: w1_load(nc.scalar, 1, 2),
                lambda: nc.scalar.dma_start(zt_sb[3][:, :, :], zt_d[3][:, :, :]),
                lambda: w2_load(nc.scalar, 1, 1),
                lambda: w1_load(nc.scalar, 2, 1),
                lambda: w1_load(nc.scalar, 2, 2),
                lambda: w2_load(nc.scalar, 2, 1),
                lambda: w1_load(nc.scalar, 3, 1),
                lambda: w1_load(nc.scalar, 3, 2),
                lambda: w2_load(nc.scalar, 3, 1),
            ]
            if has_b1:
                # the general path keeps ACT busy from slot 0 on, so there
                # are no safe interleave points; queue everything up front
                for fn in scalar_late:
                    fn()
                scalar_late = []

            # ---- PE warm-up: a few dummy matmuls on zeros keep the PE's HAM
            # activity monitor busy while the first weights stream in, so the
            # real matmul stream hits the full 2.4 GHz clock within ~1us of
            # starting instead of running its first ~3.4us at the cold
            # 1.2 GHz gate.
            phd = hpsum.tile([P, 2, 512], f32, tag="ph")
            for _ in range(9):
                nc.tensor.matmul(
                    phd[:, 0, :256],
                    dummy_sb[:, :256].rearrange("p (k m) -> p k m", k=2),
                    dummy_sb[:, :].rearrange("p (k m) -> p k m", k=2),
                    start=True, stop=True, perf_mode=DR,
                )
            nc.vector.tensor_copy(dscr_sb[:, :], phd[:, 0, :256])

            # ---- compute
            for s in range(SLOTS):
                C = int(Cs[s])
                off = int(offs[s])
                h_sb = hpool.tile([P, HT, C], f8, tag="h")
                # layer 1: h^T[ht] = relu(W1^T z^T + b1), 2 interleaved-fp8
                # matmuls (256-deep contraction each) per output tile.  Two
                # h-tiles share one 2-bank PSUM tile so the relu drains them
                # in a single (cheaper) op, alternating DVE / ACT.
                for hp in range(HT // 2):
                    ph = hpsum.tile([P, 2, 512], f32, tag="ph")
                    for i in range(2):
                        ht = 2 * hp + i
                        for j in range(DT // 2):
                            nc.tensor.matmul(
                                ph[:, i, :C],
                                w1_ap(s, ht)[:, j, :].rearrange(
                                    "p (k m) -> p k m", k=2),
                                zt_sb[s][:, 2 * j:2 * j + 2, :],
                                start=(j == 0),
                                stop=(j == DT // 2 - 1),
                                perf_mode=DR,
                            )
                    if has_b1:
                        # general path: ACT relu with per-partition bias
                        for i in range(2):
                            ht = 2 * hp + i
                            nc.scalar.activation(
                                h_sb[:, ht, :], ph[:, i, :C], Relu,
                                bias=b1_sb[:, s * HT + ht: s * HT + ht + 1],
                            )
                    elif hp % 2 == 0:
                        # DVE TT: relu(x) = max(x, 0) vs a zeros tile
                        nc.vector.tensor_max(
                            h_sb[:, 2 * hp:2 * hp + 2, :],
                            ph[:, :, :C],
                            zero_sb[:, :, :C])
                    else:
                        # split the PSUM-drain load with the ACT engine
                        nc.scalar.activation(
                            h_sb[:, 2 * hp:2 * hp + 2, :], ph[:, :, :C], Relu)
                        if scalar_late:
                            scalar_late.pop(0)()
                # layer 2: u^T[dt] = sum_ht W2[ht,dt]^T h^T[ht], 8 matmuls
                # per output tile
                u_sb = usb.tile([P, DT, C], bf16, tag="u")
                for dt in range(DT):
                    pu = upsum.tile([P, C], f32, tag="pu")
                    for t in range(HT // 2):
                        nc.tensor.matmul(
                            pu[:, :],
                            w2_ap(s, dt)[:, t, :].rearrange(
                                "p (k m) -> p k m", k=2),
                            h_sb[:, 2 * t:2 * t + 2, :],
                            start=(t == 0),
                            stop=(t == HT // 2 - 1),
                            perf_mode=DR,
                        )
                    if dt % 2 == 1:
                        nc.scalar.activation(u_sb[:, dt, :], pu[:, :], Copy)
                        if scalar_late:
                            scalar_late.pop(0)()
                    else:
                        nc.vector.tensor_copy(u_sb[:, dt, :], pu[:, :])
                # batched output DMAs: SWDGE during the run (separate queues
                # from the weight-load HWDGE rings); the last slot rides the
                # by-then-idle Sync HWDGE ring split per d-pair to cut the
                # drain tail.
                if s < SLOTS - 1:
                    nc.gpsimd.dma_start(u_d[:, :, off:off + C], u_sb[:, :, :])
                else:
                    for dt in range(DT):
                        nc.sync.dma_start(u_d[:, dt:dt + 1, off:off + C],
                                          u_sb[:, dt:dt + 1, :])
    nc.compile()
    return nc


def _get_nc(Cs, has_b1):
    key = (tuple(int(c) for c in Cs), bool(has_b1))
    if key not in _nc_cache:
        _nc_cache[key] = _build_nc(key[0], key[1])
    return _nc_cache[key]


def kernel(inp, ln_g, ln_b, wg_group, wg_inner, W1, b1, W2, b2, gln_g, gln_b):
    global LAST_RESULTS
    import jax
    import jax.numpy as jnp
    import ml_dtypes

    inp = np.asarray(inp)
    in_dtype = inp.dtype
    bf = ml_dtypes.bfloat16
    f8 = ml_dtypes.float8_e4m3

    # ---- 1. routing on host (bit-exact replica of the reference gates)
    gi, gsc, z, eis, escs = _routing(inp, ln_g, ln_b, wg_group, wg_inner)

    # token lists per (g, e)
    tok_lists, scale_lists = {}, {}
    for g in range(G):
        in_g = (gi == g).any(axis=1)
        S_g = np.nonzero(in_g)[0]
        ei, esc = eis[g], escs[g]
        for e in range(E):
            sel = ei[S_g] == e           # [|S_g|, EK]
            has = sel.any(axis=1)
            toks = S_g[has]
            w = (esc[S_g] * sel).sum(axis=1)[has]
            tok_lists[(g, e)] = toks
            scale_lists[(g, e)] = w.astype(np.float32)

    # ---- 2. balanced assignment of the 32 pairs to (core, slot)
    pairs = [(g, e) for g in range(G) for e in range(E)]
    pairs.sort(key=lambda p: -len(tok_lists[p]))
    assign = {}           # (core, slot) -> (g, e)
    Cs = []
    for s in range(SLOTS):
        rank = pairs[s * NCORES:(s + 1) * NCORES]
        Cs.append(max(CAP_GRAN, _round_up(max(len(tok_lists[p]) for p in rank), CAP_GRAN)))
        for c, p in enumerate(rank):
            assign[(c, s)] = p
    CT = int(sum(Cs))
    offs = np.concatenate([[0], np.cumsum(Cs)]).astype(int)

    # ---- 3. build per-core input maps
    W1n = np.asarray(W1, np.float32)
    W2n = np.asarray(W2, np.float32)
    b1n = np.asarray(b1, np.float32)
    b2n = np.asarray(b2, np.float32)
    z_f8 = z.astype(f8)

    def _swi(W, n_in_tiles, n_out_tiles):
        # [K, M] weight -> the PE SwInterleave stationary layout
        # [ki, out_tile, pair, 256] with columns [A127 B127 .. A0 B0]
        # (pair-interleaved, out-column-reversed).
        Wv = W.astype(f8).reshape(n_in_tiles, P, n_out_tiles, P)  # [q, ki, ot, m]
        Wp = Wv.reshape(n_in_tiles // 2, 2, P, n_out_tiles, P)    # [pair, ab, ki, ot, m]
        Wr = Wp[..., ::-1]                                        # reverse m
        # -> [ki, ot, pair, m, ab] -> interleave (m, ab) into 256
        return np.ascontiguousarray(Wr.transpose(2, 3, 0, 4, 1)).reshape(
            P, n_out_tiles, n_in_tiles // 2, 2 * P)

    in_maps = []
    for c in range(NCORES):
        # partition-major device layouts (see _build_nc)
        w1_np = np.empty((SLOTS, P, HT, DT // 2, 2 * P), f8)
        w2_np = np.empty((SLOTS, P, DT, HT // 2, 2 * P), f8)
        b1_np = np.empty((P, SLOTS * HT), np.float32)
        b1_v = b1_np.reshape(P, SLOTS, HT)
        im = {"w1": w1_np, "w2": w2_np, "b1": b1_np}
        for s in range(SLOTS):
            g, e = assign[(c, s)]
            toks = tok_lists[(g, e)]
            n = len(toks)
            # z^T tile (dt, p, c) -> [p, dt, c], one contiguous block per slot
            zt_np = np.zeros((P, DT, int(Cs[s])), f8)
            zt_np[:, :, :n] = z_f8[toks].T.reshape(DT, P, n).transpose(1, 0, 2)
            im[f"zt{s}"] = zt_np
            w1_np[s] = _swi(W1n[g, e], DT, HT)
            w2_np[s] = _swi(W2n[g, e], HT, DT)
            b1_v[:, s, :] = b1n[g, e].reshape(HT, P).T
        in_maps.append(im)

    # ---- 4. compile + run on the 8 NeuronCores
    _ensure_ntff_hook()
    from concourse.bass_utils import run_bass_kernel_spmd

    nc = _get_nc(Cs, has_b1=bool(np.any(b1n)))
    res = run_bass_kernel_spmd(
        nc, in_maps, core_ids=list(range(NCORES)),
        trace=bool(int(os.environ.get("KERNEL_TRACE", "0"))),
    )
    LAST_RESULTS = res

    # ---- 5. host combine
    moe = np.zeros((G, N, D), np.float32)
    for c in range(NCORES):
        # u layout [p, dt, CT] -> u^T[d, c] -> [CT, D]
        u = (
            np.asarray(res.results[c]["u"], np.float32)
            .transpose(1, 0, 2).reshape(D, CT).T
        )
        for s in range(SLOTS):
            g, e = assign[(c, s)]
            toks = tok_lists[(g, e)]
            n = len(toks)
            w = scale_lists[(g, e)]
            contrib = u[offs[s]:offs[s] + n] * w[:, None] + w[:, None] * b2n[g, e][None, :]
            np.add.at(moe[g], toks, contrib)

    cpu = jax.devices("cpu")[0]
    with jax.default_device(cpu):
        zj = jnp.asarray(z)
        gi_j = jnp.asarray(gi)
        gsc_j = jnp.asarray(gsc)
        gw_dense = jnp.sum(
            jax.nn.one_hot(gi_j, G, dtype=jnp.float32) * gsc_j[..., None], axis=-2
        )  # [N, G]
        out = jnp.zeros((N, D), jnp.float32)
        gg = jnp.asarray(np.asarray(gln_g, np.float32))
        gb = jnp.asarray(np.asarray(gln_b, np.float32))
        for g in range(G):
            t = zj + jnp.asarray(moe[g])
            m = jnp.mean(t, axis=-1, keepdims=True)
            tc_ = t - m
            v = jnp.mean(tc_ * tc_, axis=-1, keepdims=True)
            y = tc_ * jax.lax.rsqrt(v + EPS) * gg[g] + gb[g]
            out = out + gw_dense[:, g:g + 1] * y
        result = np.asarray(out).reshape(B, T, D) + np.asarray(inp, np.float32)

    return result.astype(in_dtype)


# revision 22
# speedup vs baseline: 1.1182x; 1.1182x over previous
"""Trainium2 kernel for nn_CustomizedMoGPositionwiseFF (moe_routing).

Strategy (expert-parallel, per the sharding hint):
  - 32 (group, expert) FFN pairs are sharded across 8 NeuronCores (4 each).
  - Routing (group top-2 gate + per-group inner top-2 gate) is computed on
    host at call time; tokens are dispatched (gathered) per expert into the
    per-core shards -- data-dependent sharding, compiled into the NEFF.
  - Each core runs both FFN matmuls + relu for its 4 experts over the tokens
    routed to them, reading each expert weight exactly once (memory regime).
    Weights and activations are shipped as fp8 e4m3 and the matmuls run in
    DoubleRow perf mode (2 fp8 weights per PE cell, 256-deep contraction):
    ~2x the bf16 PE throughput and half the weight DMA traffic.  PSUM
    accumulation stays f32; relative error vs the f32 reference ~4e-3.
  - Host applies the cheap O(N*D) combine: iw/b2 scaling, scatter-add of the
    two expert contributions per (token, group), per-group post-layernorm,
    group top-2 mixture, and the outer residual.

The kernel output layout on device is u^T = (relu(z W1 + b1) W2)^T per
dispatched token, written as [128, DT, CT] bf16 so every DMA is dense.
"""

import os
import numpy as np

# Model dims (hardcoded per the contract; match the reference problem)
B, T, D, H = 2, 1024, 512, 2048
G, E, GK, EK = 4, 8, 2, 2
EPS = 1e-5
N = B * T
P = 128
DT = D // P    # 4 d-tiles
HT = H // P    # 16 h-tiles
NCORES = 8
SLOTS = (G * E) // NCORES  # 4 experts per core
CAP_GRAN = 16              # capacity granularity (tokens)

_nc_cache = {}
LAST_RESULTS = None       # test harness can inspect (BassKernelResults)


def _ensure_ntff_hook():
    """Register antenv.axon_hooks with the ctypes NTFF profile hook if the
    container's antenv package lacks it (mirrors trn_agent_boot.trn_boot).
    Makes trace=True work; degrades to hook=None when the .so is absent."""
    try:
        from antenv.axon_hooks import get_axon_ntff_profile_hook  # noqa: F401
        return
    except ImportError:
        pass
    import sys
    import types
    import contextlib
    import ctypes

    mod = types.ModuleType("antenv.axon_hooks")
    _state = {"hook": None}

    def set_axon_ntff_profile_hook(h):
        _state["hook"] = h

    def get_axon_ntff_profile_hook():
        return _state["hook"]

    mod.set_axon_ntff_profile_hook = set_axon_ntff_profile_hook
    mod.get_axon_ntff_profile_hook = get_axon_ntff_profile_hook

    so_path = "/opt/axon/libaxon_pjrt.so"
    hook = None
    if os.path.exists(so_path):
        try:
            lib = ctypes.CDLL(so_path)
            if hasattr(lib, "axon_start_nrt_profile"):
                lib.axon_start_nrt_profile.argtypes = [
                    ctypes.POINTER(ctypes.c_int64), ctypes.c_size_t]
                lib.axon_start_nrt_profile.restype = ctypes.c_int64
                lib.axon_stop_nrt_profile.argtypes = [ctypes.c_char_p]
                lib.axon_stop_nrt_profile.restype = ctypes.c_int64

                @contextlib.contextmanager
                def _hook(output_dir, device_ids):
                    import jax
                    jax.devices()
                    if device_ids:
                        ids = (ctypes.c_int64 * len(device_ids))(*device_ids)
                        rc = lib.axon_start_nrt_profile(ids, len(device_ids))
                    else:
                        rc = lib.axon_start_nrt_profile(None, 0)
                    if rc != 0:
                        raise RuntimeError(f"axon_start_nrt_profile rc={rc}")
                    try:
                        yield
                    finally:
                        n = lib.axon_stop_nrt_profile(str(output_dir).encode())
                        print(f"ntff profile: {n} file(s) -> {output_dir}")

                hook = _hook
        except Exception:
            hook = None
    _state["hook"] = hook
    import antenv
    sys.modules["antenv.axon_hooks"] = mod
    antenv.axon_hooks = mod


def _round_up(x, m):
    return ((x + m - 1) // m) * m


def _routing(inp, ln_g, ln_b, wg_group, wg_inner):
    """Replicate the reference gating bit-for-bit on jax-cpu.

    Returns gi [N,GK] group ids, gsc [N,GK] group softmax, z [N,D] f32,
    eis/escs: per-group inner top-k ids/softmax ([N,EK] each).
    """
    import jax
    import jax.numpy as jnp

    cpu = jax.devices("cpu")[0]
    with jax.default_device(cpu):
        x = jnp.asarray(np.asarray(inp, np.float32)).reshape(-1, D)
        gl = x @ jnp.asarray(np.asarray(wg_group, np.float32))
        gv, gi = jax.lax.top_k(gl, GK)
        gsc = jax.nn.softmax(gv, axis=-1)
        m = jnp.mean(x, axis=-1, keepdims=True)
        xc = x - m
        v = jnp.mean(xc * xc, axis=-1, keepdims=True)
        z = xc * jax.lax.rsqrt(v + EPS) * jnp.asarray(np.asarray(ln_g, np.float32)) \
            + jnp.asarray(np.asarray(ln_b, np.float32))
        wgi = jnp.asarray(np.asarray(wg_inner, np.float32))
        eis, escs = [], []
        for g in range(G):
            l = z @ wgi[g]
            ev, ei = jax.lax.top_k(l, EK)
            esc = jax.nn.softmax(ev, axis=-1)
            eis.append(np.asarray(ei))
            escs.append(np.asarray(esc))
    return np.asarray(gi), np.asarray(gsc), np.asarray(z), eis, escs


def _build_nc(Cs, has_b1=False):
    """Build the SPMD Bass program for per-slot capacities Cs (uniform across cores).

    fp8 e4m3 weights + activations, DoubleRowSwInterleave matmuls (256-deep
    contraction; weights pre-interleaved on host so the stationary load is a
    dense 256-column read).
    """
    import concourse.bass as bass
    import concourse.bacc as bacc
    import concourse.tile as tile
    from concourse import mybir

    f32 = mybir.dt.float32
    bf16 = mybir.dt.bfloat16
    f8 = mybir.dt.float8e4
    DR = mybir.MatmulPerfMode.DoubleRowSwInterleave
    Relu = mybir.ActivationFunctionType.Relu
    Copy = mybir.ActivationFunctionType.Copy

    CT = int(sum(Cs))
    offs = np.concatenate([[0], np.cumsum(Cs)]).astype(int)
    CMAX = int(max(Cs))

    nc = bacc.Bacc("TRN2", target_bir_lowering=False)
    # all DRAM layouts are partition-major [128, ...] so every DMA is dense
    # contiguous lines per partition.  Weight tiles are stored as the PE's
    # SwInterleave stationary layout: per (pair, out-tile) a contiguous 256
    # columns [A127 B127 A126 B126 .. A0 B0].
    zt_d = [nc.declare_dram_parameter(f"zt{s}", [P, DT, int(Cs[s])], f8,
                                      isOutput=False) for s in range(SLOTS)]
    w1_d = nc.declare_dram_parameter("w1", [SLOTS, P, HT, DT // 2, 2 * P], f8, isOutput=False)
    w2_d = nc.declare_dram_parameter("w2", [SLOTS, P, DT, HT // 2, 2 * P], f8, isOutput=False)
    b1_d = nc.declare_dram_parameter("b1", [P, SLOTS * HT], f32, isOutput=False)
    u_d = nc.declare_dram_parameter("u", [P, DT, CT], bf16, isOutput=True)

    with tile.TileContext(nc) as tc:
        with tc.tile_pool(name="consts", bufs=1) as consts, \
             tc.tile_pool(name="hpool", bufs=2) as hpool, \
             tc.tile_pool(name="hpsum", bufs=3, space="PSUM") as hpsum, \
             tc.tile_pool(name="upsum", bufs=2, space="PSUM") as upsum, \
             tc.tile_pool(name="usb", bufs=3) as usb:

            zt_sb = [consts.tile([P, DT, int(Cs[s])], f8, tag=f"zt{s}",
                                 name=f"zt_{s}") for s in range(SLOTS)]
            b1_sb = consts.tile([P, SLOTS * HT], f32, tag="b1")
            zero_sb = consts.tile([P, 2, CMAX], f32, tag="zero")
            dummy_sb = consts.tile([P, 512], f8, tag="dummy")
            dscr_sb = consts.tile([P, 256], f8, tag="dscr")
            nc.vector.memset(dummy_sb[:, :], 0.0)
            nc.vector.memset(zero_sb[:, :, :], 0.0)
            # Weight tiles are split into independently-loaded pieces (Tile
            # tracks dependencies at tile granularity, so each piece must be
            # its own tile for compute to start before the whole weight set
            # lands).  w1: ht-ranges (slot 0 finer for the ramp); w2:
            # dt-ranges.  Each tensor's halves ride different HWDGE rings --
            # one ring sustains only ~200 GB/s, and a slot consumes weights
            # at ~280 GB/s.
            w1_splits = [[(0, 4), (4, 8), (8, 16)]] + \
                        [[(0, 8), (8, 16)]] * (SLOTS - 1)
            w2_split = [(0, 2), (2, 4)]
            w1_sb = [[consts.tile([P, b - a, DT // 2, 2 * P], f8,
                                  tag=f"w1_{s}_{a}", name=f"w1s_{s}_{a}")
                      for (a, b) in w1_splits[s]] for s in range(SLOTS)]
            w2_sb = [[consts.tile([P, b - a, HT // 2, 2 * P], f8,
                                  tag=f"w2_{s}_{a}", name=f"w2s_{s}_{a}")
                      for (a, b) in w2_split] for s in range(SLOTS)]

            def w1_ap(s, ht):
                for i, (a, b) in enumerate(w1_splits[s]):
                    if a <= ht < b:
                        return w1_sb[s][i][:, ht - a, :, :]

            def w2_ap(s, dt):
                for i, (a, b) in enumerate(w2_split):
                    if a <= dt < b:
                        return w2_sb[s][i][:, dt - a, :, :]

            def w1_load(eng, s, piece):
                a, b = w1_splits[s][piece]
                eng.dma_start(w1_sb[s][piece][:, :, :, :], w1_d[s][:, a:b, :, :])

            def w2_load(eng, s, piece):
                a, b = w2_split[piece]
                eng.dma_start(w2_sb[s][piece][:, :, :, :], w2_d[s][:, a:b, :, :])

            # ---- resident loads.  The Sync engine is free until the final
            # output DMAs, so its whole schedule is queued up front; the
            # Scalar engine also runs ACT relu/copy work, so only its first
            # few transfers are queued here and the rest are interleaved
            # into the compute stream (emitted between ACTIVATEs) to keep
            # its ring shallow.  Emission order also controls the ~10-deep
            # round-robin DMA-semaphore pool: each reused semaphore's
            # predecessor must complete before the later DMA issues, so
            # early slots' small transfers are emitted first.
            nc.sync.dma_start(zt_sb[0][:, :, :], zt_d[0][:, :, :])
            nc.scalar.dma_start(zt_sb[1][:, :, :], zt_d[1][:, :, :])
            w1_load(nc.sync, 0, 0)
            nc.scalar.dma_start(b1_sb[:, :], b1_d[:, :])
            w1_load(nc.sync, 0, 1)
            w1_load(nc.scalar, 0, 2)
            w2_load(nc.sync, 0, 0)
            w2_load(nc.scalar, 0, 1)
            nc.sync.dma_start(zt_sb[2][:, :, :], zt_d[2][:, :, :])
            w1_load(nc.scalar, 1, 1)
            w1_load(nc.sync, 1, 0)
            w2_load(nc.sync, 1, 0)
            w1_load(nc.sync, 2, 0)
            w2_load(nc.sync, 2, 0)
            w1_load(nc.sync, 3, 0)
            w2_load(nc.sync, 3, 0)
            # scalar's remaining transfers, emitted later between ACTIVATEs:
            scalar_late = [
                lambda: w2_load(nc.scalar, 1, 1),
                lambda: nc.scalar.dma_start(zt_sb[3][:, :, :], zt_d[3][:, :, :]),
                lambda: w1_load(nc.scalar, 2, 1),
                lambda: w2_load(nc.scalar, 2, 1),
                lambda: w1_load(nc.scalar, 3, 1),
                lambda: w2_load(nc.scalar, 3, 1),
            ]
            if has_b1:
                # the general path keeps ACT busy from slot 0 on, so there
                # are no safe interleave points; queue everything up front
                for fn in scalar_late:
                    fn()
                scalar_late = []

            # ---- PE warm-up: a few dummy matmuls on zeros keep the PE's HAM
            # activity monitor busy while the first weights stream in, so the
            # real matmul stream hits the full 2.4 GHz clock within ~1us of
            # starting instead of running its first ~3.4us at the cold
            # 1.2 GHz gate.
            phd = hpsum.tile([P, 2, 512], f32, tag="ph")
            for _ in range(9):
                nc.tensor.matmul(
                    phd[:, 0, :256],
                    dummy_sb[:, :256].rearrange("p (k m) -> p k m", k=2),
                    dummy_sb[:, :].rearrange("p (k m) -> p k m", k=2),
                    start=True, stop=True, perf_mode=DR,
                )
            nc.vector.tensor_copy(dscr_sb[:, :], phd[:, 0, :256])

            # ---- compute
            for s in range(SLOTS):
                C = int(Cs[s])
                off = int(offs[s])
                h_sb = hpool.tile([P, HT, C], f8, tag="h")
                # layer 1: h^T[ht] = relu(W1^T z^T + b1), 2 interleaved-fp8
                # matmuls (256-deep contraction each) per output tile.  Two
                # h-tiles share one 2-bank PSUM tile so the relu drains them
                # in a single (cheaper) op, alternating DVE / ACT.
                for hp in range(HT // 2):
                    ph = hpsum.tile([P, 2, 512], f32, tag="ph")
                    for i in range(2):
                        ht = 2 * hp + i
                        for j in range(DT // 2):
                            nc.tensor.matmul(
                                ph[:, i, :C],
                                w1_ap(s, ht)[:, j, :].rearrange(
                                    "p (k m) -> p k m", k=2),
                                zt_sb[s][:, 2 * j:2 * j + 2, :],
                                start=(j == 0),
                                stop=(j == DT // 2 - 1),
                                perf_mode=DR,
                            )
                    if has_b1:
                        # general path: ACT relu with per-partition bias
                        for i in range(2):
                            ht = 2 * hp + i
                            nc.scalar.activation(
                                h_sb[:, ht, :], ph[:, i, :C], Relu,
                                bias=b1_sb[:, s * HT + ht: s * HT + ht + 1],
                            )
                    elif hp % 2 == 0:
                        # DVE TT: relu(x) = max(x, 0) vs a zeros tile
                        nc.vector.tensor_max(
                            h_sb[:, 2 * hp:2 * hp + 2, :],
                            ph[:, :, :C],
                            zero_sb[:, :, :C])
                    else:
                        # split the PSUM-drain load with the ACT engine
                        nc.scalar.activation(
                            h_sb[:, 2 * hp:2 * hp + 2, :], ph[:, :, :C], Relu)
                        if scalar_late:
                            scalar_late.pop(0)()
                # layer 2: u^T[dt] = sum_ht W2[ht,dt]^T h^T[ht], 8 matmuls
                # per output tile
                u_sb = usb.tile([P, DT, C], bf16, tag="u")
                for dt in range(DT):
                    pu = upsum.tile([P, C], f32, tag="pu")
                    for t in range(HT // 2):
                        nc.tensor.matmul(
                            pu[:, :],
                            w2_ap(s, dt)[:, t, :].rearrange(
                                "p (k m) -> p k m", k=2),
                            h_sb[:, 2 * t:2 * t + 2, :],
                            start=(t == 0),
                            stop=(t == HT // 2 - 1),
                            perf_mode=DR,
                        )
                    if dt % 2 == 1:
                        nc.scalar.activation(u_sb[:, dt, :], pu[:, :], Copy)
                        if scalar_late:
                            scalar_late.pop(0)()
                    else:
                        nc.vector.tensor_copy(u_sb[:, dt, :], pu[:, :])
                # batched output DMAs: SWDGE during the run (separate queues
                # from the weight-load HWDGE rings); the last slot rides the
                # by-then-idle Sync HWDGE ring split per d-pair to cut the
                # drain tail.
                if s < SLOTS - 1:
                    nc.gpsimd.dma_start(u_d[:, :, off:off + C], u_sb[:, :, :])
                else:
                    for dt in range(DT):
                        nc.sync.dma_start(u_d[:, dt:dt + 1, off:off + C],
                                          u_sb[:, dt:dt + 1, :])
    nc.compile()
    return nc


def _get_nc(Cs, has_b1):
    key = (tuple(int(c) for c in Cs), bool(has_b1))
    if key not in _nc_cache:
        _nc_cache[key] = _build_nc(key[0], key[1])
    return _nc_cache[key]


def kernel(inp, ln_g, ln_b, wg_group, wg_inner, W1, b1, W2, b2, gln_g, gln_b):
    global LAST_RESULTS
    import jax
    import jax.numpy as jnp
    import ml_dtypes

    inp = np.asarray(inp)
    in_dtype = inp.dtype
    bf = ml_dtypes.bfloat16
    f8 = ml_dtypes.float8_e4m3

    # ---- 1. routing on host (bit-exact replica of the reference gates)
    gi, gsc, z, eis, escs = _routing(inp, ln_g, ln_b, wg_group, wg_inner)

    # token lists per (g, e)
    tok_lists, scale_lists = {}, {}
    for g in range(G):
        in_g = (gi == g).any(axis=1)
        S_g = np.nonzero(in_g)[0]
        ei, esc = eis[g], escs[g]
        for e in range(E):
            sel = ei[S_g] == e           # [|S_g|, EK]
            has = sel.any(axis=1)
            toks = S_g[has]
            w = (esc[S_g] * sel).sum(axis=1)[has]
            tok_lists[(g, e)] = toks
            scale_lists[(g, e)] = w.astype(np.float32)

    # ---- 2. balanced assignment of the 32 pairs to (core, slot)
    pairs = [(g, e) for g in range(G) for e in range(E)]
    pairs.sort(key=lambda p: -len(tok_lists[p]))
    assign = {}           # (core, slot) -> (g, e)
    Cs = []
    for s in range(SLOTS):
        rank = pairs[s * NCORES:(s + 1) * NCORES]
        Cs.append(max(CAP_GRAN, _round_up(max(len(tok_lists[p]) for p in rank), CAP_GRAN)))
        for c, p in enumerate(rank):
            assign[(c, s)] = p
    CT = int(sum(Cs))
    offs = np.concatenate([[0], np.cumsum(Cs)]).astype(int)

    # ---- 3. build per-core input maps
    W1n = np.asarray(W1, np.float32)
    W2n = np.asarray(W2, np.float32)
    b1n = np.asarray(b1, np.float32)
    b2n = np.asarray(b2, np.float32)
    z_f8 = z.astype(f8)

    def _swi(W, n_in_tiles, n_out_tiles):
        # [K, M] weight -> the PE SwInterleave stationary layout
        # [ki, out_tile, pair, 256] with columns [A127 B127 .. A0 B0]
        # (pair-interleaved, out-column-reversed).
        Wv = W.astype(f8).reshape(n_in_tiles, P, n_out_tiles, P)  # [q, ki, ot, m]
        Wp = Wv.reshape(n_in_tiles // 2, 2, P, n_out_tiles, P)    # [pair, ab, ki, ot, m]
        Wr = Wp[..., ::-1]                                        # reverse m
        # -> [ki, ot, pair, m, ab] -> interleave (m, ab) into 256
        return np.ascontiguousarray(Wr.transpose(2, 3, 0, 4, 1)).reshape(
            P, n_out_tiles, n_in_tiles // 2, 2 * P)

    in_maps = []
    for c in range(NCORES):
        # partition-major device layouts (see _build_nc)
        w1_np = np.empty((SLOTS, P, HT, DT // 2, 2 * P), f8)
        w2_np = np.empty((SLOTS, P, DT, HT // 2, 2 * P), f8)
        b1_np = np.empty((P, SLOTS * HT), np.float32)
        b1_v = b1_np.reshape(P, SLOTS, HT)
        im = {"w1": w1_np, "w2": w2_np, "b1": b1_np}
        for s in range(SLOTS):
            g, e = assign[(c, s)]
            toks = tok_lists[(g, e)]
            n = len(toks)
            # z^T tile (dt, p, c) -> [p, dt, c], one contiguous block per slot
            zt_np = np.zeros((P, DT, int(Cs[s])), f8)
            zt_np[:, :, :n] = z_f8[toks].T.reshape(DT, P, n).transpose(1, 0, 2)
            im[f"zt{s}"] = zt_np
            w1_np[s] = _swi(W1n[g, e], DT, HT)
            w2_np[s] = _swi(W2n[g, e], HT, DT)
            b1_v[:, s, :] = b1n[g, e].reshape(HT, P).T
        in_maps.append(im)

    # ---- 4. compile + run on the 8 NeuronCores
    _ensure_ntff_hook()
    from concourse.bass_utils import run_bass_kernel_spmd

    nc = _get_nc(Cs, has_b1=bool(np.any(b1n)))
    res = run_bass_kernel_spmd(
        nc, in_maps, core_ids=list(range(NCORES)),
        trace=bool(int(os.environ.get("KERNEL_TRACE", "0"))),
    )
    LAST_RESULTS = res

    # ---- 5. host combine
    moe = np.zeros((G, N, D), np.float32)
    for c in range(NCORES):
        # u layout [p, dt, CT] -> u^T[d, c] -> [CT, D]
        u = (
            np.asarray(res.results[c]["u"], np.float32)
            .transpose(1, 0, 2).reshape(D, CT).T
        )
        for s in range(SLOTS):
            g, e = assign[(c, s)]
            toks = tok_lists[(g, e)]
            n = len(toks)
            w = scale_lists[(g, e)]
            contrib = u[offs[s]:offs[s] + n] * w[:, None] + w[:, None] * b2n[g, e][None, :]
            np.add.at(moe[g], toks, contrib)

    cpu = jax.devices("cpu")[0]
    with jax.default_device(cpu):
        zj = jnp.asarray(z)
        gi_j = jnp.asarray(gi)
        gsc_j = jnp.asarray(gsc)
        gw_dense = jnp.sum(
            jax.nn.one_hot(gi_j, G, dtype=jnp.float32) * gsc_j[..., None], axis=-2
        )  # [N, G]
        out = jnp.zeros((N, D), jnp.float32)
        gg = jnp.asarray(np.asarray(gln_g, np.float32))
        gb = jnp.asarray(np.asarray(gln_b, np.float32))
        for g in range(G):
            t = zj + jnp.asarray(moe[g])
            m = jnp.mean(t, axis=-1, keepdims=True)
            tc_ = t - m
            v = jnp.mean(tc_ * tc_, axis=-1, keepdims=True)
            y = tc_ * jax.lax.rsqrt(v + EPS) * gg[g] + gb[g]
            out = out + gw_dense[:, g:g + 1] * y
        result = np.asarray(out).reshape(B, T, D) + np.asarray(inp, np.float32)

    return result.astype(in_dtype)
